# revision 27
# baseline (speedup 1.0000x reference)
"""Trainium2 Bass kernel for the DSCNMP GNN (2x GINConv + pooling + MLP head).

Self-contained: takes full (unsharded) inputs, shards nodes/edges across the
8 NeuronCores internally, runs one SPMD Bass program via
bass_utils.run_bass_kernel_spmd, and returns the full [G, O] output.

Sharding strategy:
  - Nodes partitioned contiguously across 8 cores; each edge owned by the
    core of its dst node. Small MLP/BN weights replicated.
  - HOST-SIDE BALANCED PARTITION: within each core, nodes are assigned to
    4 quarters (= gather-table chunks, randomly balanced), and within each
    quarter snake-packed by in-degree into dst groups (6x512-wide + 1x128)
    and 128-node tiles, so per-(chunk, dst-group) edge-cell sizes are nearly
    equal. SPMD needs one instruction stream for all cores, so each cell's
    slot count is the max over cores - balancing makes that max small.
  - Per-edge x1 gathers via gpsimd dma_gather from the AllGathered x1 table
    (4 chunk tables to satisfy the int16 index range).
  - Segment-sum on the TensorEngine: per 128-edge slot a one-hot matrix
    S[e, j] = (dst_e == group_base + j) is built on DVE and agg^T
    accumulates in PSUM as land^T @ S per 512-wide dst group.
  - conv1 runs per-quarter so AllGather(q) fires ~q/4 into conv1; gathers
    for chunk c start right after AllGather(c) completes, overlapping the
    rest of conv1 and the other collectives.
  - Pooled graph embeddings AllReduced; graph-level head replicated.
"""

import numpy as np
import ml_dtypes

N_FULL, E_FULL, G_FULL, C_DIM, H_DIM, O_DIM = 100000, 600000, 1000, 2, 128, 10
HC_DIM = H_DIM // 2
NCORES = 8
NCHUNK = 4           # quarters = int16 gather-index chunking
CALL_SLOTS = 16      # 128-edge slots per dma_gather call
EPS = 1e-5

_CACHE = {}


def _pack_idx16(flat):
    """[j%16, j//16] int16 packing, replicated across the 8 Q7 groups."""
    total = len(flat)
    assert total % 16 == 0
    out = flat.reshape(total // 16, 16).T.astype(np.int16)
    return np.tile(out, (8, 1))


def _preprocess(pos, edge_index, batch, N, E, G):
    NL = N // NCORES            # 12500 nodes per core
    NLP = 12800                 # padded local slots
    NT = NLP // 128             # 100 tiles
    QL = NLP // NCHUNK          # 3200 slots per quarter
    TPQ = NT // NCHUNK          # 25 tiles per quarter
    NGRP = 7 * NCHUNK           # dst groups per core (6 big + 1 small per q)

    pos = np.asarray(pos, np.float32)
    src = np.asarray(edge_index[0], np.int64)
    dst = np.asarray(edge_index[1], np.int64)
    batch = np.asarray(batch, np.int64)
    assert N % NCORES == 0 and NL * NCORES == N

    rng = np.random.default_rng(12345)
    deg_in = np.bincount(dst, minlength=N)

    # ---- balanced node -> local slot assignment ----
    # quarter: random balanced (3125 nodes per (core, quarter))
    # within quarter: snake by in-degree into 25 tiles of 125 nodes
    #   (tiles 0..23 of the quarter form 6 big 512-wide dst groups,
    #    tile 24 is the small 128-wide group)
    NPQ = NL // NCHUNK          # 3125 nodes per (core, quarter)
    NPT = NPQ // TPQ            # 125 nodes per tile
    slot_of = np.empty(N, np.int64)
    for k in range(NCORES):
        nodes = np.arange(k * NL, (k + 1) * NL)
        perm = rng.permutation(NL)
        for q in range(NCHUNK):
            nn = nodes[perm[q * NPQ:(q + 1) * NPQ]]
            order = np.argsort(-deg_in[nn], kind="stable")
            sn = nn[order]
            # snake rows of 25 across tiles
            tl = np.empty(NPQ, np.int64)
            row = np.arange(NPQ) // TPQ
            col = np.arange(NPQ) % TPQ
            fwd = (row % 2) == 0
            tl[fwd] = col[fwd]
            tl[~fwd] = TPQ - 1 - col[~fwd]
            # position within tile = running count
            posi = np.zeros(NPQ, np.int64)
            cnt = np.zeros(TPQ, np.int64)
            for i in range(NPQ):
                posi[i] = cnt[tl[i]]
                cnt[tl[i]] += 1
            assert cnt.max() <= NPT
            slot_of[sn] = (q * TPQ + tl) * 128 + posi

    dslot = slot_of[dst]        # local dst slot (0..12799)
    ecore = dst // NL
    ksrc = src // NL
    q_src = slot_of[src] // QL              # chunk of src
    grow = ksrc * QL + (slot_of[src] % QL)  # row within chunk table
    assert grow.max() < QL * NCORES <= 32767 + 1

    t_loc = dslot // 128                    # local dst tile (0..99)
    qd = t_loc // TPQ
    tq = t_loc % TPQ
    gg = np.where(tq < 24, 7 * qd + tq // 4, 7 * qd + 6)   # dst group id 0..27
    # group base slot and width
    g_ids = np.arange(NGRP)
    g_q = g_ids // 7
    g_i = g_ids % 7
    g_base = g_q * QL + np.where(g_i < 6, g_i * 512, 3072)
    g_width = np.where(g_i < 6, 512, 128)

    # ---- conv1 cells: per dst tile ----
    loads1 = np.zeros((NCORES, NT), np.int64)
    for k in range(NCORES):
        m = ecore == k
        loads1[k] = np.bincount(t_loc[m], minlength=NT)
    SLC1 = np.maximum(1, -(-loads1.max(axis=0) // 128))     # slots per tile
    off1 = np.concatenate([[0], np.cumsum(SLC1)])           # slot offsets
    NS1 = int(off1[-1])
    STREAM1 = NS1 * 128

    # ---- conv2 cells: (chunk of src, dst group) ----
    loads2 = np.zeros((NCORES, NCHUNK, NGRP), np.int64)
    for k in range(NCORES):
        m = ecore == k
        cc = q_src[m] * NGRP + gg[m]
        loads2[k] = np.bincount(cc, minlength=NCHUNK * NGRP).reshape(
            NCHUNK, NGRP)
    slots2 = np.maximum(0, -(-loads2.max(axis=0) // 128))   # [NCHUNK, NGRP]
    off2 = np.zeros((NCHUNK, NGRP + 1), np.int64)
    chunk_base = np.zeros(NCHUNK + 1, np.int64)
    for c in range(NCHUNK):
        off2[c, 1:] = np.cumsum(slots2[c])
        chunk_base[c + 1] = chunk_base[c] + off2[c, -1]
    NS2 = int(chunk_base[-1])
    STREAM2 = NS2 * 128

    # ---- per-core data arrays ----
    pos_nm = np.zeros((NCORES, 128, NT * C_DIM), np.float32)
    batch_rel = np.full((NCORES, 128, NT), -5.0, np.float32)
    g0 = np.zeros(NCORES, np.int64)
    gwin_need = 0
    for k in range(NCORES):
        nodes = np.arange(k * NL, (k + 1) * NL)
        j = slot_of[nodes]
        pos_nm[k][j % 128, (j // 128) * C_DIM + 0] = pos[nodes, 0]
        pos_nm[k][j % 128, (j // 128) * C_DIM + 1] = pos[nodes, 1]
        g0[k] = batch[nodes[0]]
        rel = batch[nodes] - g0[k]
        batch_rel[k][j % 128, j // 128] = rel.astype(np.float32)
        gwin_need = max(gwin_need, int(rel.max()) + 1)
    GWIN = min(512, max(128, -(-gwin_need // 32) * 32))
    assert gwin_need <= GWIN <= 512
    WG = -(-(G + GWIN) // 256) * 256

    posE = np.zeros((NCORES, 128, NS1 * C_DIM), np.float32)
    dwc1 = np.full((NCORES, 128, NS1), -5.0, np.float32)
    gidx = np.zeros((NCORES, 128, STREAM2 // 16), np.int16)
    dwc2 = np.full((NCORES, 128, NS2), -5.0, np.float32)

    for k in range(NCORES):
        m = ecore == k
        gs, ds_, qs, ggs = grow[m], dslot[m], q_src[m], gg[m]
        ps_ = pos[src[m]]
        # conv1 stream: cells by tile, sorted by dslot
        o = np.argsort(ds_, kind="stable")  # tile-major since tile=ds_//128
        ds1 = ds_[o]
        pe1 = ps_[o]
        t1 = ds1 // 128
        # position within cell
        cellpos = np.arange(len(ds1)) - np.searchsorted(t1, t1)
        spos = off1[t1] * 128 + cellpos
        lane = spos % 128
        sl = spos // 128
        dwc1[k][lane, sl] = (ds1 - t1 * 128).astype(np.float32)
        posE[k][lane, sl * C_DIM + 0] = pe1[:, 0]
        posE[k][lane, sl * C_DIM + 1] = pe1[:, 1]
        # conv2 stream: cells by (chunk, group), sorted by dslot
        key = (qs * NGRP + ggs) * 16384 + ds_
        o2 = np.argsort(key, kind="stable")
        ds2 = ds_[o2]
        g2 = ggs[o2]
        q2 = qs[o2]
        gr2 = gs[o2]
        cell2 = q2 * NGRP + g2
        cellpos2 = np.arange(len(ds2)) - np.searchsorted(cell2, cell2)
        base_slot = chunk_base[q2] + off2[q2, g2]
        spos2 = base_slot * 128 + cellpos2
        lane2 = spos2 % 128
        sl2 = spos2 // 128
        gi = np.zeros(STREAM2, np.int64)
        gi[sl2 * 128 + lane2] = gr2
        gidx[k] = _pack_idx16(gi)
        dwc2[k][lane2, sl2] = (ds2 - g_base[g2]).astype(np.float32)
        assert (ds2 - g_base[g2] >= 0).all()
        assert (ds2 - g_base[g2] < g_width[g2]).all()

    groff = np.zeros((NCORES, 1, 2), np.int32)
    groff[:, 0, 0] = g0
    assert (g0 + GWIN <= WG).all()

    # host-precomputed one-hot S matrices (streamed from DRAM on device)
    iota128 = np.arange(128, dtype=np.float32)
    iota512 = np.arange(512, dtype=np.float32)
    S1t = (dwc1[:, :, :, None] == iota128).astype(ml_dtypes.bfloat16).reshape(
        NCORES, 128, NS1 * 128)
    S2t = (dwc2[:, :, :, None] == iota512).astype(ml_dtypes.bfloat16).reshape(
        NCORES, 128, NS2 * 512)

    dims = dict(N=N, E=E, G=G, NL=NL, NLP=NLP, NT=NT, QL=QL, TPQ=TPQ,
                NGRP=NGRP, NS1=NS1, NS2=NS2, STREAM1=STREAM1, STREAM2=STREAM2,
                GWIN=GWIN, WG=WG,
                SLC1=tuple(int(x) for x in SLC1),
                slots2=tuple(int(x) for x in slots2.flat),
                g_base=tuple(int(x) for x in g_base),
                g_width=tuple(int(x) for x in g_width))
    arrays = dict(posE=posE, pos_nm=pos_nm, batch_rel=batch_rel,
                  gidx=gidx, S1t=S1t, S2t=S2t, groff=groff, slot_of=slot_of)
    return dims, arrays


def _build_program(dims):
    import contextlib
    import concourse.bass as bass
    import concourse.bacc as bacc
    import concourse.mybir as mybir
    import concourse.tile as tile
    from concourse import library_config
    from concourse.masks import make_identity

    f32 = mybir.dt.float32
    bf16 = mybir.dt.bfloat16
    f16 = mybir.dt.float16
    i16 = mybir.dt.int16
    i32 = mybir.dt.int32
    AF = mybir.ActivationFunctionType
    ALU = mybir.AluOpType

    NLP, NT, QL, TPQ = dims["NLP"], dims["NT"], dims["QL"], dims["TPQ"]
    NGRP = dims["NGRP"]
    NS1, NS2 = dims["NS1"], dims["NS2"]
    GWIN, WG, G = dims["GWIN"], dims["WG"], dims["G"]
    SLC1 = dims["SLC1"]
    slots2 = np.array(dims["slots2"], np.int64).reshape(NCHUNK, NGRP)
    g_base = dims["g_base"]
    g_width = dims["g_width"]
    off1 = np.concatenate([[0], np.cumsum(SLC1)])
    off2 = np.zeros((NCHUNK, NGRP + 1), np.int64)
    chunk_base = np.zeros(NCHUNK + 1, np.int64)
    for c in range(NCHUNK):
        off2[c, 1:] = np.cumsum(slots2[c])
        chunk_base[c + 1] = chunk_base[c] + off2[c, -1]

    nc = bacc.Bacc("TRN2", target_bir_lowering=False, debug=False,
                   enable_asserts=True, num_devices=NCORES)

    def din(name, shape, dt=f32):
        return nc.dram_tensor(name, list(shape), dt, kind="ExternalInput")

    posE_d = din("posE", [128, NS1 * C_DIM], bf16)
    pos_nm_d = din("pos_nm", [128, NT * C_DIM], bf16)
    batch_rel_d = din("batch_rel", [128, NT])
    gidx_d = din("gidx", [128, NS2 * 8], i16)
    S1t_d = din("S1t", [128, NS1 * 128], bf16)
    S2t_d = din("S2t", [128, NS2 * 512], bf16)
    groff_d = din("groff", [1, 2], i32)
    iota_d = din("iota", [128, 512])

    wnames = {}
    for nm, shp in [("W1a", [C_DIM, H_DIM]), ("W1b", [H_DIM, H_DIM]),
                    ("W2a", [H_DIM, H_DIM]), ("W2b", [H_DIM, H_DIM]),
                    ("Wf1", [C_DIM, H_DIM]), ("Wf2", [H_DIM, H_DIM]),
                    ("Wc1", [H_DIM, HC_DIM]), ("Wc2", [HC_DIM, O_DIM])]:
        wnames[nm] = din(nm, shp)
    vecs = {}
    for nm in ["b1a", "b1b", "b2a", "b2b", "bf1", "bf2",
               "n1_g", "n1_b", "n1_rm", "n1_rv", "n2_g", "n2_b", "n2_rm", "n2_rv",
               "f1_g", "f1_b", "f1_rm", "f1_rv", "f2_g", "f2_b", "f2_rm", "f2_rv"]:
        vecs[nm] = din(nm, [H_DIM, 1])
    for nm in ["bc1", "gc", "bec", "rmc", "rvc", "a_prelu_v"]:
        vecs[nm] = din(nm, [HC_DIM, 1])
    vecs["bc2"] = din("bc2", [O_DIM, 1])

    out_d = nc.dram_tensor("out", [G, O_DIM], f32, kind="ExternalOutput")

    with tile.TileContext(nc) as tc:
        nc.gpsimd.load_library(library_config.mlp)
        ctx = contextlib.ExitStack()
        with ctx:
            dram = ctx.enter_context(tc.tile_pool(name="dram", bufs=1, space="DRAM"))
            pconst = ctx.enter_context(tc.tile_pool(name="const", bufs=1))
            pbig = ctx.enter_context(tc.tile_pool(name="big", bufs=1))
            pland = ctx.enter_context(tc.tile_pool(name="land", bufs=5))
            psmall = ctx.enter_context(tc.tile_pool(name="small", bufs=4))
            ps1p = ctx.enter_context(tc.tile_pool(name="s1p", bufs=2))
            ps2p = ctx.enter_context(tc.tile_pool(name="s2p", bufs=4))
            pgr = ctx.enter_context(tc.tile_pool(name="gr", bufs=1))
            ph1 = ctx.enter_context(tc.tile_pool(name="h1w", bufs=2))
            ppsum = ctx.enter_context(tc.tile_pool(name="psum", bufs=2, space="PSUM"))
            pseg1 = ctx.enter_context(tc.tile_pool(name="psum_seg1", bufs=2, space="PSUM"))
            pseg2 = ctx.enter_context(tc.tile_pool(name="psum_seg2", bufs=2, space="PSUM"))
            ppool = ctx.enter_context(tc.tile_pool(name="psum_acc", bufs=2, space="PSUM"))

            cc_in = [dram.tile([QL, H_DIM], bf16, tag="cc_in", name=f"cc_in{q}",
                                bufs=NCHUNK) for q in range(NCHUNK)]
            cc_out = [dram.tile([QL * NCORES, H_DIM], bf16, tag="cc_out",
                                name=f"cc_out{q}", addr_space="Shared",
                                bufs=NCHUNK) for q in range(NCHUNK)]
            ar1_in = dram.tile([H_DIM + C_DIM, WG], f32, tag="ar1_in")
            ar1_out = dram.tile([H_DIM + C_DIM, WG], f32, tag="ar1_out", addr_space="Shared")
            ar2_in = dram.tile([H_DIM, WG], f32, tag="ar2_in")
            ar2_out = dram.tile([H_DIM, WG], f32, tag="ar2_out", addr_space="Shared")

            def load_const(dr, shape, dt=f32):
                t = pconst.tile(shape, dt, tag=dr.name + "_sb")
                nc.sync.dma_start(out=t[:], in_=dr.ap())
                return t

            W = {k: load_const(v, v.shape) for k, v in wnames.items()}
            V = {k: load_const(v, v.shape) for k, v in vecs.items()}
            pos_nm = load_const(pos_nm_d, [128, NT * C_DIM], bf16)
            posE = load_const(posE_d, [128, NS1 * C_DIM], bf16)
            batch_rel = load_const(batch_rel_d, [128, NT])
            iota = load_const(iota_d, [128, 512])
            gidx = load_const(gidx_d, [128, NS2 * 8], i16)

            groff = load_const(groff_d, [1, 2], i32)

            ident = pconst.tile([128, 128], f32, tag="ident")
            make_identity(nc, ident[:])
            iota_bf = pconst.tile([128, 512], bf16, tag="iota_bf")
            nc.vector.tensor_copy(iota_bf[:], iota[:])
            ident_bf = pconst.tile([128, 128], bf16, tag="ident_bf")
            nc.vector.tensor_copy(ident_bf[:], ident[:])
            W1a_bf = pconst.tile([C_DIM, H_DIM], bf16, tag="W1a_bf")
            nc.vector.tensor_copy(W1a_bf[:], W["W1a"][:])
            W1b_bf = pconst.tile([H_DIM, H_DIM], bf16, tag="W1b_bf")
            nc.vector.tensor_copy(W1b_bf[:], W["W1b"][:])
            W2b_bf = pconst.tile([H_DIM, H_DIM], bf16, tag="W2b_bf")
            nc.vector.tensor_copy(W2b_bf[:], W["W2b"][:])

            def bn_vec(g, b, rm, rv, P, nm):
                a = pconst.tile([P, 1], f32, tag=f"bn_a_{nm}")
                c = pconst.tile([P, 1], f32, tag=f"bn_c_{nm}")
                nc.vector.tensor_scalar(a[:], rv[:], EPS, None, ALU.add)
                nc.scalar.activation(a[:], a[:], AF.Sqrt)
                nc.vector.reciprocal(a[:], a[:])
                nc.vector.tensor_tensor(a[:], a[:], g[:], op=ALU.mult)
                nc.vector.tensor_tensor(c[:], rm[:], a[:], op=ALU.mult)
                nc.vector.tensor_tensor(c[:], b[:], c[:], op=ALU.subtract)
                return a, c
            a1, c1 = bn_vec(V["n1_g"], V["n1_b"], V["n1_rm"], V["n1_rv"], H_DIM, "n1")
            a2, c2 = bn_vec(V["n2_g"], V["n2_b"], V["n2_rm"], V["n2_rv"], H_DIM, "n2")
            af1, cf1 = bn_vec(V["f1_g"], V["f1_b"], V["f1_rm"], V["f1_rv"], H_DIM, "f1")
            af2, cf2 = bn_vec(V["f2_g"], V["f2_b"], V["f2_rm"], V["f2_rv"], H_DIM, "f2")
            acl, ccl = bn_vec(V["gc"], V["bec"], V["rmc"], V["rvc"], HC_DIM, "cls")

            # persistent big buffers
            zbuf = pbig.tile([128, NLP], bf16, tag="A")      # z1 then z2 (bf16)
            xT = pbig.tile([128, NLP], f32, tag="B")         # x1T -> h2T -> x2T
            xnm = pbig.tile([128, NT * H_DIM], bf16, tag="NM")  # x1nm then x2nm

            posE_v = posE[:].rearrange("p (s c) -> p s c", c=C_DIM)

            # S-matrix builders (one-hot columns via DVE is_equal)
            s1cache = {}

            def get_s1(s):
                b = s // 8
                if b not in s1cache:
                    n = min(8, NS1 - b * 8)
                    S = ps1p.tile([128, 8 * 128], bf16, tag="S1")
                    nc.sync.dma_start(
                        out=S[:, 0:n * 128],
                        in_=S1t_d.ap()[:, b * 1024:b * 1024 + n * 128])
                    s1cache[b] = S
                return s1cache[b], (s % 8) * 128

            s2cache = {}
            # width of the dst group each conv2 slot belongs to
            s2_width = np.empty(NS2, np.int64)
            for c_ in range(NCHUNK):
                for g_ in range(NGRP):
                    for sl_ in range(int(slots2[c_, g_])):
                        s2_width[int(chunk_base[c_]) + int(off2[c_, g_]) + sl_] \
                            = g_width[g_]

            def get_s2(s):
                b = s // 4
                if b not in s2cache:
                    n = min(4, NS2 - b * 4)
                    S = ps2p.tile([128, 4 * 512], bf16, tag="S2")
                    nc.sync.dma_start(
                        out=S[:, 0:n * 512],
                        in_=S2t_d.ap()[:, b * 2048:b * 2048 + n * 512])
                    s2cache[b] = S
                return s2cache[b], (s % 4) * 512

            # zero-fill collective input buffers (outside the graph window)
            zrow = pgr.tile([H_DIM, 256], f32, tag="zrow")
            nc.vector.memset(zrow[:], 0.0)
            for zc in range(0, WG, 256):
                nc.sync.dma_start(out=ar1_in[0:H_DIM, zc:zc + 256], in_=zrow[:])
                nc.sync.dma_start(out=ar1_in[H_DIM:, zc:zc + 256],
                                  in_=zrow[0:C_DIM, :])
                nc.sync.dma_start(out=ar2_in[:, zc:zc + 256], in_=zrow[:])

            # ================= conv1, per quarter =================
            h1w = {}
            ps_pos = ppool.tile([C_DIM, GWIN], f32, tag="acc")
            ps_x1 = ppool.tile([128, GWIN], f32, tag="acc")
            for q in range(NCHUNK):
                tiles = range(q * TPQ, (q + 1) * TPQ)
                for t in tiles:
                    ps = pseg1.tile([C_DIM, 128], f32, tag="seg1")
                    for sl in range(SLC1[t]):
                        s = int(off1[t]) + sl
                        S, soff = get_s1(s)
                        nc.tensor.matmul(ps[:], posE_v[:, s, :],
                                         S[:, soff:soff + 128],
                                         start=(sl == 0), stop=False)
                    nc.tensor.matmul(ps[:], pos_nm[:, t * C_DIM:(t + 1) * C_DIM],
                                     ident_bf[:], start=(SLC1[t] == 0), stop=True)
                    tq = t % TPQ
                    w = min(tq // 4, 6)
                    sub = (tq % 4) * 128 if tq < 24 else 0
                    if (q, w) not in h1w:
                        wd = 512 if w < 6 else 128
                        h1w[(q, w)] = ph1.tile([C_DIM, wd], bf16, tag="h1w",
                                               name=f"h1w{q}_{w}")
                    nc.scalar.copy(h1w[(q, w)][:, sub:sub + 128], ps[:])
                # conv1 MLP over this quarter (windows = dst groups)
                for w in range(7):
                    wd = 512 if w < 6 else 128
                    c0 = q * QL + w * 512
                    ps = ppsum.tile([H_DIM, 512], f32, tag="work")
                    nc.tensor.matmul(ps[:, 0:wd], W1a_bf[:], h1w[(q, w)][:],
                                     start=True, stop=True)
                    nc.scalar.activation(zbuf[:, c0:c0 + wd], ps[:, 0:wd],
                                         AF.Relu, bias=V["b1a"][:], scale=1.0)
                for w in range(7):
                    wd = 512 if w < 6 else 128
                    c0 = q * QL + w * 512
                    ps = ppsum.tile([H_DIM, 512], f32, tag="work")
                    nc.tensor.matmul(ps[:, 0:wd], W1b_bf[:], zbuf[:, c0:c0 + wd],
                                     start=True, stop=True)
                    nc.scalar.activation(ps[:, 0:wd], ps[:, 0:wd], AF.Relu,
                                         bias=V["b1b"][:], scale=1.0)
                    nc.scalar.activation(xT[:, c0:c0 + wd], ps[:, 0:wd],
                                         AF.Identity, bias=c1[:], scale=a1[:])
                # x1 node-major (bf16) for the gather table + pooling
                for t in tiles:
                    pt = ppsum.tile([128, 512], f32, tag="work")
                    nc.tensor.transpose(pt[:, 0:128], xT[:, t * 128:(t + 1) * 128],
                                        ident[:])
                    nc.scalar.copy(xnm[:, t * 128:(t + 1) * 128], pt[:, 0:128])
                nc.scalar.dma_start(
                    out=cc_in[q][:].rearrange("(s p) f -> p s f", p=128),
                    in_=xnm[:, q * TPQ * H_DIM:(q + 1) * TPQ * H_DIM].rearrange(
                        "p (s f) -> p s f", f=H_DIM))
                # pools of pos and x1
                for t in tiles:
                    B = psmall.tile([128, GWIN], bf16, tag="B")
                    nc.vector.tensor_scalar(B[:], iota_bf[:, 0:GWIN],
                                            batch_rel[:, t:t + 1], None,
                                            ALU.is_equal)
                    nc.tensor.matmul(ps_pos[:], pos_nm[:, t * C_DIM:(t + 1) * C_DIM],
                                     B[:], start=(t == 0), stop=(t == NT - 1))
                    nc.tensor.matmul(ps_x1[:], xnm[:, t * 128:(t + 1) * 128], B[:],
                                     start=(t == 0), stop=(t == NT - 1))
                if q == 0:
                    # prebuild the first S2 one-hots so chunk-0 seg matmuls
                    # don't queue behind the rest of conv1's DVE work
                    for s in range(0, 16, 4):
                        get_s2(s)
            arin_pos = pgr.tile([C_DIM, GWIN], f32, tag="arin_p")
            nc.scalar.copy(arin_pos[:], ps_pos[:])
            arin_x1 = pgr.tile([H_DIM, GWIN], f32, tag="arin", bufs=2)
            nc.scalar.copy(arin_x1[:], ps_x1[:])

            # ================= conv2: gathers + segment sum =================
            ncalls = [int(-(-int(off2[c, -1]) // CALL_SLOTS)) for c in range(NCHUNK)]

            lands = {}

            def issue_gather(c, w):
                ns = min(CALL_SLOTS, int(off2[c, -1]) - w * CALL_SLOTS)
                nidx = ns * 128
                base = (int(chunk_base[c]) + w * CALL_SLOTS) * 128
                land = pland.tile([128, CALL_SLOTS, H_DIM], bf16, tag="land")
                nc.gpsimd.dma_gather(
                    land[:, 0:ns, :], cc_out[c][:],
                    gidx[:, base // 16:(base + nidx) // 16],
                    nidx, nidx, H_DIM, single_packet=False)
                lands[(c, w)] = land

            def g_mlp(lhsT_w, rhs, out, bias, bn, P=H_DIM, relu=True):
                for w in range(-(-WG // 512)):
                    c0 = w * 512
                    cw = min(512, WG - c0)
                    ps = ppsum.tile([P, 512], f32, tag="work")
                    nc.tensor.matmul(ps[:P, :cw], lhsT_w[:], rhs[:, c0:c0 + cw],
                                     start=True, stop=True)
                    fn = AF.Relu if relu else AF.Identity
                    nc.scalar.activation(ps[:P, :cw], ps[:P, :cw], fn,
                                         bias=bias[:], scale=1.0)
                    if bn is not None:
                        a_, c_ = bn
                        nc.scalar.activation(out[:, c0:c0 + cw], ps[:P, :cw],
                                             AF.Identity, bias=c_[:],
                                             scale=a_[:])
                    else:
                        nc.scalar.copy(out[:, c0:c0 + cw], ps[:P, :cw])

            def finish_group(g):
                # conv2 MLP + x2 transposes + pool for dst group g
                q, gi = g // 7, g % 7
                wd = g_width[g]
                gb = g_base[g]
                ps = ppsum.tile([H_DIM, 512], f32, tag="work")
                nc.tensor.matmul(ps[:, 0:wd], W["W2a"][:], xT[:, gb:gb + wd],
                                 start=True, stop=True)
                nc.scalar.activation(zbuf[:, gb:gb + wd], ps[:, 0:wd],
                                     AF.Relu, bias=V["b2a"][:], scale=1.0)
                ps = ppsum.tile([H_DIM, 512], f32, tag="work")
                nc.tensor.matmul(ps[:, 0:wd], W2b_bf[:], zbuf[:, gb:gb + wd],
                                 start=True, stop=True)
                nc.scalar.activation(ps[:, 0:wd], ps[:, 0:wd], AF.Relu,
                                     bias=V["b2b"][:], scale=1.0)
                nc.scalar.activation(xT[:, gb:gb + wd], ps[:, 0:wd],
                                     AF.Identity, bias=c2[:], scale=a2[:])
                tls = ([25 * q + 4 * gi + i for i in range(4)] if gi < 6
                       else [25 * q + 24])
                for t in tls:
                    pt = ppsum.tile([128, 512], f32, tag="work")
                    nc.tensor.transpose(pt[:, 0:128], xT[:, t * 128:(t + 1) * 128],
                                        ident[:])
                    nc.scalar.copy(xnm[:, t * 128:(t + 1) * 128], pt[:, 0:128])
                    B = psmall.tile([128, GWIN], bf16, tag="B")
                    nc.vector.tensor_scalar(B[:], iota_bf[:, 0:GWIN],
                                            batch_rel[:, t:t + 1], None,
                                            ALU.is_equal)
                    nc.tensor.matmul(ps_x2[:], xnm[:, t * 128:(t + 1) * 128], B[:],
                                     start=(t == 0), stop=(t == NT - 1))

            ps_x2 = ppool.tile([128, GWIN], f32, tag="acc")
            ar1x = pgr.tile([H_DIM, WG], f32, tag="arbig", bufs=2)
            ar1p = pgr.tile([C_DIM, WG], f32, tag="ar1p")
            x0g = pgr.tile([H_DIM, WG], f32, tag="g_x0g")
            x1g = pgr.tile([H_DIM, WG], f32, tag="g_x1g", bufs=2)
            gtmp = pgr.tile([H_DIM, WG], f32, tag="g_tmp")

            def issue_ag(c):
                nc.gpsimd.collective_compute(
                    "AllGather", mybir.AluOpType.bypass,
                    ins=[cc_in[c].opt()], outs=[cc_out[c].opt()],
                    replica_groups=[list(range(NCORES))])

            issue_ag(0)
            for c in range(NCHUNK):
                ag_next = False
                issued = 0
                for g in range(NGRP):
                    ns = int(slots2[c, g])
                    if ns:
                        last_slot = int(off2[c, g]) + ns - 1
                        need = min(last_slot // CALL_SLOTS + 1, ncalls[c] - 1)
                        while issued <= need:
                            issue_gather(c, issued)
                            issued += 1
                        if issued >= 4 and not ag_next and c + 1 < NCHUNK:
                            issue_ag(c + 1)
                            ag_next = True
                        wd = g_width[g]
                        gb = g_base[g]
                        ps = pseg2.tile([H_DIM, 512], f32, tag="seg2")
                        for sl in range(ns):
                            s = int(off2[c, g]) + sl
                            call, sic = s // CALL_SLOTS, s % CALL_SLOTS
                            S, soff = get_s2(int(chunk_base[c]) + s)
                            nc.tensor.matmul(ps[:, 0:wd],
                                             lands[(c, call)][:, sic, :],
                                             S[:, soff:soff + wd],
                                             start=(sl == 0), stop=False)
                        nc.tensor.matmul(ps[:, 0:wd], ident[:],
                                         xT[:, gb:gb + wd],
                                         start=False, stop=True)
                        nc.scalar.copy(xT[:, gb:gb + wd], ps[:, 0:wd])
                    if c == NCHUNK - 1:
                        # all chunks flushed for group g -> finish it
                        finish_group(g)
                while issued < ncalls[c]:
                    issue_gather(c, issued)
                    issued += 1
                if c == 1:
                    # AllReduce 1 (pooled pos + x1), hidden under the gathers
                    with nc.gpsimd.register("g0r") as g0r:
                        nc.gpsimd.reg_load(g0r, groff[0:1, 0:1])
                        sv = nc.gpsimd.snap(g0r, min_val=0, max_val=WG - GWIN)
                    nc.gpsimd.dma_start(out=ar1_in[0:H_DIM, bass.ds(sv, GWIN)],
                                        in_=arin_x1[:])
                    nc.gpsimd.dma_start(out=ar1_in[H_DIM:, bass.ds(sv, GWIN)],
                                        in_=arin_pos[:])
                    nc.gpsimd.collective_compute(
                        "AllReduce", mybir.AluOpType.add,
                        ins=[ar1_in.opt()], outs=[ar1_out.opt()],
                        replica_groups=[list(range(NCORES))])
                    nc.scalar.dma_start(out=ar1x[:], in_=ar1_out[0:H_DIM, :])
                    nc.scalar.dma_start(out=ar1p[:], in_=ar1_out[H_DIM:, :])
                if c == 2:
                    # graph-stage pieces that only need AR1 (x0g, x1g)
                    g_mlp(W["Wf1"], ar1p[:], x0g, V["bf1"], (af1, cf1))
                    nc.vector.tensor_tensor(gtmp[:], x0g[:], ar1x[:], op=ALU.add)
                    g_mlp(W["Wf2"], gtmp, x1g, V["bf2"], (af2, cf2))

            arin2 = pgr.tile([H_DIM, GWIN], f32, tag="arin", bufs=2)
            nc.scalar.copy(arin2[:], ps_x2[:])
            with nc.gpsimd.register("g0r2") as g0r2:
                nc.gpsimd.reg_load(g0r2, groff[0:1, 0:1])
                sv2 = nc.gpsimd.snap(g0r2, min_val=0, max_val=WG - GWIN)
            nc.gpsimd.dma_start(out=ar2_in[:, bass.ds(sv2, GWIN)], in_=arin2[:])
            nc.gpsimd.collective_compute(
                "AllReduce", mybir.AluOpType.add,
                ins=[ar2_in.opt()], outs=[ar2_out.opt()],
                replica_groups=[list(range(NCORES))])

            # ================= graph stage (tail) =================
            nc.vector.tensor_tensor(gtmp[:], x0g[:], x1g[:], op=ALU.add)
            ar2t = pgr.tile([H_DIM, WG], f32, tag="arbig", bufs=2)
            nc.scalar.dma_start(out=ar2t[:], in_=ar2_out[:])
            nc.vector.tensor_tensor(gtmp[:], gtmp[:], ar2t[:], op=ALU.add)
            x2g = pgr.tile([H_DIM, WG], f32, tag="arbig", bufs=2)
            g_mlp(W["Wf2"], gtmp, x2g, V["bf2"], (af2, cf2))

            hcls = pgr.tile([HC_DIM, WG], f32, tag="g_tmp")
            g_mlp(W["Wc1"], x2g, hcls, V["bc1"], (acl, ccl), P=HC_DIM, relu=False)
            hneg = pgr.tile([HC_DIM, WG], f32, tag="g_x1g", bufs=2)
            nc.vector.tensor_scalar(hneg[:], hcls[:], V["a_prelu_v"][:], None,
                                    ALU.mult)
            nc.vector.tensor_tensor(hcls[:], hcls[:], hneg[:], op=ALU.max)
            outT = pgr.tile([O_DIM, WG], f32, tag="g_x1g", bufs=2)
            g_mlp(W["Wc2"], hcls, outT, V["bc2"], None, P=O_DIM, relu=False)

            ngt = -(-G // 128)
            onm = pgr.tile([128, ngt * O_DIM], f32, tag="onm")
            for j in range(ngt):
                pt = ppsum.tile([128, 512], f32, tag="work")
                nc.tensor.transpose(pt[:, 0:O_DIM], outT[:, j * 128:(j + 1) * 128],
                                    ident[0:O_DIM, 0:O_DIM])
                nc.scalar.copy(onm[:, j * O_DIM:(j + 1) * O_DIM], pt[:, 0:O_DIM])
            nfull = G // 128
            if nfull:
                nc.sync.dma_start(
                    out=out_d.ap()[0:nfull * 128, :].rearrange(
                        "(s p) o -> p s o", p=128),
                    in_=onm[:, :nfull * O_DIM].rearrange(
                        "p (s o) -> p s o", o=O_DIM))
            rem = G - nfull * 128
            if rem:
                nc.sync.dma_start(out=out_d.ap()[nfull * 128:G, :],
                                  in_=onm[0:rem, nfull * O_DIM:(nfull + 1) * O_DIM])

    nc.compile()
    return nc


def _build_in_maps(inputs, dims, arrays):
    import ml_dtypes
    f = lambda x: np.ascontiguousarray(np.asarray(x, np.float32))
    col = lambda x: f(x).reshape(-1, 1)
    shared = {
        "iota": np.tile(np.arange(512, dtype=np.float32), (128, 1)),
        "W1a": f(inputs["W1a"]), "W1b": f(inputs["W1b"]),
        "W2a": f(inputs["W2a"]), "W2b": f(inputs["W2b"]),
        "Wf1": f(inputs["Wf1"]), "Wf2": f(inputs["Wf2"]),
        "Wc1": f(inputs["Wc1"]), "Wc2": f(inputs["Wc2"]),
        "b1a": col(inputs["b1a"]), "b1b": col(inputs["b1b"]),
        "b2a": col(inputs["b2a"]), "b2b": col(inputs["b2b"]),
        "bf1": col(inputs["bf1"]), "bf2": col(inputs["bf2"]),
        "bc1": col(inputs["bc1"]), "bc2": col(inputs["bc2"]),
        "gc": col(inputs["gc"]), "bec": col(inputs["bec"]),
        "rmc": col(inputs["rmc"]), "rvc": col(inputs["rvc"]),
        "a_prelu_v": np.full((HC_DIM, 1),
                             np.float32(np.asarray(inputs["a_prelu"]))),
    }
    for pfx in ["n1_", "n2_", "f1_", "f2_"]:
        for sfx in ["g", "b", "rm", "rv"]:
            shared[pfx + sfx] = col(inputs[pfx + sfx])
    in_maps = []
    for k in range(NCORES):
        m = dict(shared)
        m["posE"] = arrays["posE"][k].astype(ml_dtypes.bfloat16)
        m["pos_nm"] = arrays["pos_nm"][k].astype(ml_dtypes.bfloat16)
        m["batch_rel"] = arrays["batch_rel"][k]
        m["gidx"] = arrays["gidx"][k]
        m["S1t"] = arrays["S1t"][k]
        m["S2t"] = arrays["S2t"][k]
        m["groff"] = arrays["groff"][k]
        in_maps.append(m)
    return in_maps


def _get_compiled(pos, edge_index, batch, N, E, G):
    dims, arrays = _preprocess(pos, edge_index, batch, N, E, G)
    key = tuple(sorted((k, v) for k, v in dims.items()))
    if key not in _CACHE:
        _CACHE[key] = _build_program(dims)
    return _CACHE[key], dims, arrays


def kernel(**inputs):
    from concourse.bass_utils import run_bass_kernel_spmd
    pos = np.asarray(inputs["pos"])
    ei = np.asarray(inputs["edge_index"])
    batch = np.asarray(inputs["batch"])
    nc, dims, arrays = _get_compiled(pos, ei, batch, pos.shape[0],
                                     ei.shape[1], G_FULL)
    in_maps = _build_in_maps(inputs, dims, arrays)
    res = run_bass_kernel_spmd(nc, in_maps, list(range(NCORES)))
    return np.asarray(res.results[0]["out"], np.float32)
